# revision 30
# baseline (speedup 1.0000x reference)
"""EpisodicMemory Trainium2 kernel.

Data-parallel over batch across 8 NeuronCores (128 batch rows per core).
Per-core program (SPMD, weights baked into the NEFF as inline constants):

  Scoring: G[b,s] = sigmoid(fc2(tanh(fc1(feat)))),
    feat = [C*Q, C*prev_M, |C-Q|, |C-prev_M|], built in fp32/fp32r in a
    feature-on-partitions layout; fc1 is an fp32r matmul with N=512.
    Scoring work is interleaved into the scan (one group of 4 sentences
    spread over 4 scan steps, two groups of lookahead) so it fills
    engine idle time created by the scan's serial dependence.

  Scan (gated GRU over s): per step the PE computes
    [C_s; h] @ [W_ih; W_hh].T in fp32r (r/z fused; n split because r
    multiplies only the hidden half), state h kept [batch-part, H-free],
    h transposed on the PE each step to feed the next matmul. Update:
    h_new = h + g*(1-z)*(n - h), with per-partition gate scalars.
    The r-path and the chain tail are split into column halves so the
    serial chain pipelines across engines; subs/abs/gating run on the
    otherwise-idle GPSIMD engine.
"""
import numpy as np

H = 512
SH = 120
B = 1024
S = 64
NCORES = 8
BPC = B // NCORES  # 128
KH = H // 128      # 4
G3 = 3 * H
SGRP = 4
NGRP = S // SGRP   # 16
HHALF = H // 2     # 256

_CACHE = {}


def _build(Wt, F1t, F2t, fc1_b, fc2_b, b_ih, b_hh):
    import concourse.bass as bass
    import concourse.tile as tile
    from concourse import bacc, mybir

    FP32 = mybir.dt.float32
    FP32R = mybir.dt.float32r
    BF16 = mybir.dt.bfloat16
    U16 = mybir.dt.uint16
    U32 = mybir.dt.uint32
    AF = mybir.ActivationFunctionType
    OP = mybir.AluOpType

    use_gru_bias = bool(np.any(b_ih != 0) or np.any(b_hh != 0))
    use_fc1_bias = bool(np.any(fc1_b != 0))
    fc2_bias = float(np.asarray(fc2_b).reshape(-1)[0])

    nc = bacc.Bacc("TRN2", target_bir_lowering=False, debug=False,
                   num_devices=NCORES)

    c_t = nc.dram_tensor("c_t", [S, KH, 128, BPC], FP32R, kind="ExternalInput")
    q_t = nc.dram_tensor("q_t", [KH, 128, BPC], FP32, kind="ExternalInput")
    m_t = nc.dram_tensor("m_t", [KH, 128, BPC], FP32, kind="ExternalInput")
    out = nc.dram_tensor("out", [BPC, H], FP32, kind="ExternalOutput")

    wt_d = nc.inline_tensor(Wt, name="wt")              # [8,128,1536] fp32
    f1_d = nc.inline_tensor(F1t.astype(np.float32), name="f1t")
    f2_d = nc.inline_tensor(F2t.astype(np.float32), name="f2t")
    id_d = nc.inline_tensor(np.eye(128, dtype=np.float32), name="ident")
    if use_fc1_bias:
        f1b_d = nc.inline_tensor(fc1_b.reshape(SH, 1).astype(np.float32),
                                 name="f1b")
    if use_gru_bias:
        ones_d = nc.inline_tensor(np.ones((1, 128), np.float32), name="ones1")
        bx_d = nc.inline_tensor(b_ih.reshape(1, G3).astype(np.float32),
                                name="bx")
        bh_d = nc.inline_tensor(b_hh.reshape(1, G3).astype(np.float32),
                                name="bh")

    from contextlib import ExitStack
    with tile.TileContext(nc) as tc:
        with ExitStack() as ctx:
            cpool = ctx.enter_context(tc.tile_pool(name="const", bufs=1))
            hpool = ctx.enter_context(tc.tile_pool(name="state", bufs=3))
            p1sb = ctx.enter_context(tc.tile_pool(name="p1sb", bufs=2))
            fpool = ctx.enter_context(tc.tile_pool(name="feat", bufs=10))
            gpool = ctx.enter_context(tc.tile_pool(name="gtile", bufs=4))
            gdram = ctx.enter_context(tc.tile_pool(name="gdram", bufs=4, space="DRAM"))
            cspool = ctx.enter_context(tc.tile_pool(name="cs", bufs=3))
            cbpool = ctx.enter_context(tc.tile_pool(name="cbs", bufs=3))
            htpool = ctx.enter_context(tc.tile_pool(name="ht", bufs=2))
            ew = ctx.enter_context(tc.tile_pool(name="ew", bufs=2))
            ew1 = ctx.enter_context(tc.tile_pool(name="ew1", bufs=2))
            ps_r = ctx.enter_context(tc.tile_pool(name="ps_r", bufs=2, space="PSUM"))
            ps_z = ctx.enter_context(tc.tile_pool(name="ps_z", bufs=1, space="PSUM"))
            ps_n = ctx.enter_context(tc.tile_pool(name="ps_n", bufs=2, space="PSUM"))
            ps_h = ctx.enter_context(tc.tile_pool(name="ps_h", bufs=1, space="PSUM"))
            ps_t = ctx.enter_context(tc.tile_pool(name="ps_t", bufs=1, space="PSUM"))
            ps_f = ctx.enter_context(tc.tile_pool(name="ps_f", bufs=1, space="PSUM"))
            # ---- constants ----
            wt = cpool.tile([128, 8, G3], FP32R, tag="wt")
            nc.sync.dma_start(
                wt[:], wt_d.ap().rearrange("k h g -> h k g").bitcast(FP32R))
            f1t = cpool.tile([128, 16, SH], FP32R, tag="f1t")
            nc.sync.dma_start(
                f1t[:], f1_d.ap().rearrange("k h o -> h k o").bitcast(FP32R))
            f2t = cpool.tile([SH, 1], FP32R, tag="f2t")
            nc.sync.dma_start(f2t[:], f2_d.ap().bitcast(FP32R))
            idt = cpool.tile([128, 128], FP32, tag="idt")
            nc.sync.dma_start(idt[:], id_d.ap())
            qm = cpool.tile([128, KH, BPC], FP32, tag="qm")
            nc.sync.dma_start(qm[:], q_t.ap().rearrange("k h b -> h k b"))
            mm = cpool.tile([128, KH, BPC], FP32, tag="mm")
            nc.sync.dma_start(mm[:], m_t.ap().rearrange("k h b -> h k b"))
            # materialized broadcast of Q/M over the 4 sentences of a group
            qrep = cpool.tile([128, KH, SGRP, BPC], FP32, tag="qrep")
            mrep = cpool.tile([128, KH, SGRP, BPC], FP32, tag="mrep")
            for k in range(KH):
                nc.vector.tensor_copy(
                    qrep[:, k],
                    qm[:, k].unsqueeze(1).broadcast_to([128, SGRP, BPC]))
                nc.vector.tensor_copy(
                    mrep[:, k],
                    mm[:, k].unsqueeze(1).broadcast_to([128, SGRP, BPC]))
            if use_fc1_bias:
                f1b = cpool.tile([SH, 1], FP32, tag="f1b")
                nc.sync.dma_start(f1b[:], f1b_d.ap())
            if use_gru_bias:
                onest = cpool.tile([1, 128], FP32R, tag="ones1")
                nc.sync.dma_start(onest[:], ones_d.ap().bitcast(FP32R))
                bxt = cpool.tile([1, G3], FP32R, tag="bx")
                nc.sync.dma_start(bxt[:], bx_d.ap().bitcast(FP32R))
                bht = cpool.tile([1, G3], FP32R, tag="bh")
                nc.sync.dma_start(bht[:], bh_d.ap().bitcast(FP32R))

            # ---- scoring group machinery (interleaved into the scan) ----
            grp_state = {}   # gi -> dict with cgb, pps, g4, ng4

            def load_group(gi):
                cgb = cbpool.tile([128, KH, SGRP, BPC], FP32, tag="cgb")
                s0 = gi * SGRP
                for k in range(KH):
                    nc.sync.dma_start(
                        cgb[:, k],
                        c_t.ap().bitcast(FP32)[s0:s0 + SGRP, k].rearrange("s h b -> h s b"))
                grp_state[gi] = {"cgb": cgb}

            fc1_pending = []   # (gi, q, [feat tiles]) awaiting fc1 matmuls

            def emit_feat_quarter(gi, q):
                """Emit feat k-tiles for quarter q (DVE/GPSIMD/ACT only)."""
                st = grp_state[gi]
                cgb = st["cgb"]
                fks = []
                # subs go first so the GPSIMD finishes them early and the
                # ACT abs ops can run in ACT idle instead of blocking the
                # next step's chain sigmoids; muls run on the DVE.
                for k in (8 + q, 12 + q, q, 4 + q):
                    kc = k % KH
                    which = k // KH
                    fk = fpool.tile([128, SGRP, BPC], FP32R, tag="feat")
                    cgk = cgb[:, kc]
                    rep = qrep if which in (0, 2) else mrep
                    if which <= 1:
                        nc.vector.tensor_tensor(
                            fk[:], cgk, rep[:, kc], OP.mult)
                    else:
                        dt_ = p1sb.tile([128, SGRP, BPC], FP32, tag="dtmp")
                        nc.gpsimd.tensor_tensor(
                            dt_[:], cgk, rep[:, kc], OP.subtract)
                        nc.scalar.activation(fk[:], dt_[:], AF.Abs)
                    fks.append((k, fk))
                fks.sort(key=lambda kf: kf[0])
                fc1_pending.append((gi, q, fks))

            def flush_fc1():
                while fc1_pending:
                    gi, q, fks = fc1_pending.pop(0)
                    st = grp_state[gi]
                    if q == 0:
                        st["pps"] = ps_f.tile([SH, SGRP * BPC], FP32,
                                              tag="pps", name="pps")
                    pps = st["pps"]
                    for i, (k, fk) in enumerate(fks):
                        nc.tensor.matmul(pps[:], f1t[:, k], fk[:],
                                         start=(i == 0 and q == 0),
                                         stop=(i == 3 and q == 3))
                    if q == 3:
                        finish_group(gi)

            def finish_group(gi):
                st = grp_state[gi]
                pps = st["pps"]
                h1 = p1sb.tile([SH, SGRP * BPC], FP32R, tag="h1")
                if use_fc1_bias:
                    nc.scalar.activation(h1[:], pps[:], AF.Tanh,
                                         bias=f1b[:, 0:1])
                else:
                    nc.scalar.activation(h1[:], pps[:], AF.Tanh)
                nc.tensor.matmul(pps[0:1, :], f2t[:], h1[:],
                                 start=True, stop=True)
                gt = gpool.tile([1, SGRP * BPC], FP32, tag="gt")
                nc.scalar.activation(gt[:], pps[0:1, :], AF.Sigmoid,
                                     bias=fc2_bias)
                gd = gdram.tile([SGRP, BPC], FP32, tag="gd")
                nc.sync.dma_start(gd[:], gt[:])
                g4 = gpool.tile([BPC, SGRP], FP32, tag="g4")
                nc.sync.dma_start(g4[:], gd[:].rearrange("s b -> b s"))
                st["g4"] = g4
                del st["cgb"], st["pps"]

            # prologue: groups 0 and 1 fully
            for gi in (0, 1):
                load_group(gi)
                for q in range(4):
                    emit_feat_quarter(gi, q)
                flush_fc1()

            # initial state: two column halves
            h_half = []
            for hf in range(2):
                t0 = hpool.tile([BPC, HHALF], FP32, tag=f"h{hf}", name=f"h{hf}")
                nc.vector.memset(t0[:], 0.0)
                h_half.append(t0)

            for s in range(S):
                # --- C_s load (fp32r, for matmul lhsT) ---
                cst = cspool.tile([128, KH, BPC], FP32R, tag="cs")
                nc.sync.dma_start(
                    cst[:], c_t.ap()[s].rearrange("k h b -> h k b"))

                # --- x-part matmuls (no h dependence) ---
                pr = ps_r.tile([BPC, H], FP32, tag="pr")
                pz = ps_z.tile([BPC, H], FP32, tag="pz")
                pxn = ps_n.tile([BPC, H], FP32, tag="pxn")
                n_r = KH if s == 0 else 2 * KH
                for k in range(KH):
                    nc.tensor.matmul(
                        pr[:], cst[:, k], wt[:, k, 0:H],
                        start=(k == 0),
                        stop=(k == n_r - 1 and not use_gru_bias))
                for k in range(KH):
                    nc.tensor.matmul(
                        pz[:], cst[:, k], wt[:, k, H:2 * H],
                        start=(k == 0),
                        stop=(k == n_r - 1 and not use_gru_bias))
                for k in range(KH):
                    nc.tensor.matmul(
                        pxn[:], cst[:, k], wt[:, k, 2 * H:G3],
                        start=(k == 0),
                        stop=(k == KH - 1 and not use_gru_bias))
                if use_gru_bias:
                    nc.tensor.matmul(pr[:], onest[:], bxt[:, 0:H],
                                     start=False, stop=(s == 0))
                    if s == 0:
                        nc.tensor.matmul(pr[:], onest[:], bht[:, 0:H],
                                         start=False, stop=False)
                    nc.tensor.matmul(pz[:], onest[:], bxt[:, H:2 * H],
                                     start=False, stop=(s == 0))
                    if s == 0:
                        nc.tensor.matmul(pz[:], onest[:], bht[:, H:2 * H],
                                         start=False, stop=False)
                    nc.tensor.matmul(pxn[:], onest[:], bxt[:, 2 * H:G3],
                                     start=False, stop=True)

                # --- h-dependent matmuls: per state-half chains
                #     transpose -> evac -> accumulate, so the first half's
                #     matmuls issue while the second half is still in the
                #     elementwise tail of the previous step.
                if s > 0:
                    pt = ps_t.tile([128, H], FP32, tag="pt")
                    ht = htpool.tile([128, KH, BPC], FP32R, tag="ht")
                    phn = ps_h.tile([BPC, H], FP32, tag="phn")

                    def mm_h(psum, k, c0, start=False, stop=False):
                        nc.tensor.matmul(
                            psum[:], ht[:, k], wt[:, KH + k, c0:c0 + H],
                            start=start, stop=stop)

                    # h1 half first (its tail is the loop-carried critical
                    # path); h0's transposes interleave right after pr-k2/k3
                    # so they don't queue behind the whole h1 matmul block.
                    for k in (2, 3):
                        nc.tensor.transpose(
                            pt[:, k * 128:(k + 1) * 128],
                            h_half[1][:, (k - 2) * 128:(k - 1) * 128], idt[:])
                    nc.scalar.activation(
                        ht[:, 2:4],
                        pt[:, 256:512].rearrange("h (k b) -> h k b", k=2),
                        AF.Copy)
                    mm_h(pr, 2, 0)
                    mm_h(pr, 3, 0)
                    for k in (0, 1):
                        nc.tensor.transpose(
                            pt[:, k * 128:(k + 1) * 128],
                            h_half[0][:, k * 128:(k + 1) * 128], idt[:])
                    mm_h(phn, 2, 2 * H, start=True)
                    mm_h(phn, 3, 2 * H)
                    mm_h(pz, 2, H)
                    mm_h(pz, 3, H)
                    nc.scalar.activation(
                        ht[:, 0:2],
                        pt[:, 0:256].rearrange("h (k b) -> h k b", k=2),
                        AF.Copy)
                    mm_h(pr, 0, 0)
                    mm_h(pr, 1, 0, stop=not use_gru_bias)
                    mm_h(phn, 0, 2 * H)
                    mm_h(phn, 1, 2 * H, stop=not use_gru_bias)
                    mm_h(pz, 0, H)
                    mm_h(pz, 1, H, stop=not use_gru_bias)
                    if use_gru_bias:
                        nc.tensor.matmul(pr[:], onest[:], bht[:, 0:H],
                                         start=False, stop=True)
                        nc.tensor.matmul(pz[:], onest[:], bht[:, H:2 * H],
                                         start=False, stop=True)
                        nc.tensor.matmul(phn[:], onest[:], bht[:, 2 * H:G3],
                                         start=False, stop=True)

                # fc1 matmuls for feat emitted LAST step (inputs long ready)
                flush_fc1()

                # --- elementwise update (r-path and tail in column halves) ---
                gst = grp_state[s // SGRP]
                g4 = gst["g4"]
                j = s % SGRP

                r_sb = ew.tile([BPC, H], FP32, tag="r")
                w_sb = ew.tile([BPC, H], FP32, tag="w")
                for hf in (1, 0):
                    c0 = hf * HHALF
                    nc.scalar.activation(r_sb[:, c0:c0 + HHALF],
                                         pr[:, c0:c0 + HHALF], AF.Sigmoid)

                # n-path: h1 half first (it carries the loop dependence)
                n_h = [None, None]
                for hf in (1, 0):
                    c0 = hf * HHALF
                    n_hf = ew1.tile([BPC, HHALF], FP32, tag=f"n{hf}", name=f"n{hf}")
                    if s > 0:
                        tn = ew1.tile([BPC, HHALF], FP32, tag=f"tn{hf}", name=f"tn{hf}")
                        nc.vector.tensor_tensor(
                            tn[:], r_sb[:, c0:c0 + HHALF],
                            phn[:, c0:c0 + HHALF], OP.mult)
                        tn2 = ew1.tile([BPC, HHALF], FP32, tag=f"tn2{hf}", name=f"tn2{hf}")
                        nc.vector.tensor_tensor(
                            tn2[:], tn[:], pxn[:, c0:c0 + HHALF], OP.add)
                        nc.scalar.activation(n_hf[:], tn2[:], AF.Tanh)
                    else:
                        nc.scalar.activation(n_hf[:], pxn[:, c0:c0 + HHALF],
                                             AF.Tanh)
                    n_h[hf] = n_hf
                # w = 1 - z = sigmoid(-pz); needed only by the tails, so it
                # is emitted after the n-path to keep tanh off the ACT queue
                for hf in (1, 0):
                    c0 = hf * HHALF
                    nc.scalar.activation(w_sb[:, c0:c0 + HHALF],
                                         pz[:, c0:c0 + HHALF], AF.Sigmoid,
                                         scale=-1.0)
                # tails: h1 on the DVE (fast, critical), h0 on the GPSIMD.
                # The final gate is split per 128-col quarter so each PE
                # transpose fires as soon as its quarter of h_new lands.
                new_h = [None, None]
                for hf in (1, 0):
                    c0 = hf * HHALF
                    eng = nc.vector if hf == 1 else nc.gpsimd
                    t_hf = ew1.tile([BPC, HHALF], FP32, tag=f"t{hf}", name=f"t{hf}")
                    eng.tensor_sub(t_hf[:], n_h[hf][:], h_half[hf][:])
                    u_hf = ew1.tile([BPC, HHALF], FP32, tag=f"u{hf}", name=f"u{hf}")
                    eng.tensor_mul(u_hf[:], w_sb[:, c0:c0 + HHALF], t_hf[:])
                    nh = hpool.tile([BPC, HHALF], FP32, tag=f"h{hf}", name=f"h{hf}")
                    for qq in range(2):
                        nc.vector.scalar_tensor_tensor(
                            nh[:, qq * 128:(qq + 1) * 128],
                            u_hf[:, qq * 128:(qq + 1) * 128],
                            g4[:, j:j + 1],
                            h_half[hf][:, qq * 128:(qq + 1) * 128],
                            OP.mult, OP.add)
                    new_h[hf] = nh
                h_half = new_h

                # --- interleaved scoring feat work (emitted after the
                #     chain-critical ops so it fills engine idle time) ---
                gi = s // SGRP + 2
                q = s % SGRP
                if gi <= NGRP - 1:
                    if q == 0:
                        load_group(gi)
                    emit_feat_quarter(gi, q)

            flush_fc1()
            nc.sync.dma_start(out.ap()[:, 0:HHALF], h_half[0][:])
            nc.sync.dma_start(out.ap()[:, HHALF:H], h_half[1][:])

    nc.compile()
    return nc


def _prep(C, Q, prev_M, fc1_w, fc2_w, W_ih, W_hh):
    """Host-side sharding + layout transforms."""
    Wt = np.concatenate([
        np.ascontiguousarray(W_ih.T).reshape(KH, 128, G3),
        np.ascontiguousarray(W_hh.T).reshape(KH, 128, G3),
    ], axis=0).astype(np.float32)
    F1t = np.ascontiguousarray(fc1_w.T).reshape(16, 128, SH).astype(np.float32)
    F2t = np.ascontiguousarray(fc2_w.T).astype(np.float32)  # [120, 1]

    in_maps = []
    for c in range(NCORES):
        lo, hi = c * BPC, (c + 1) * BPC
        c_tr = np.ascontiguousarray(C[lo:hi].transpose(1, 2, 0)).reshape(
            S, KH, 128, BPC).astype(np.float32)
        q_tr = np.ascontiguousarray(Q[lo:hi, 0].T).reshape(
            KH, 128, BPC).astype(np.float32)
        m_tr = np.ascontiguousarray(prev_M[lo:hi, 0].T).reshape(
            KH, 128, BPC).astype(np.float32)
        in_maps.append({"c_t": c_tr, "q_t": q_tr, "m_t": m_tr})
    return Wt, F1t, F2t, in_maps


def kernel(C, Q, prev_M, fc1_w, fc1_b, fc2_w, fc2_b, W_ih, W_hh, b_ih, b_hh):
    from concourse.bass_utils import run_bass_kernel_spmd

    C = np.asarray(C, dtype=np.float32)
    Q = np.asarray(Q, dtype=np.float32)
    prev_M = np.asarray(prev_M, dtype=np.float32)
    Wt, F1t, F2t, in_maps = _prep(C, Q, prev_M,
                                  np.asarray(fc1_w, np.float32),
                                  np.asarray(fc2_w, np.float32),
                                  np.asarray(W_ih, np.float32),
                                  np.asarray(W_hh, np.float32))

    key = (Wt.tobytes(), F1t.tobytes(), F2t.tobytes(),
           np.asarray(fc1_b).tobytes(), np.asarray(fc2_b).tobytes(),
           np.asarray(b_ih).tobytes(), np.asarray(b_hh).tobytes())
    kh = hash(key)
    if kh not in _CACHE:
        _CACHE[kh] = _build(Wt, F1t, F2t,
                            np.asarray(fc1_b, np.float32),
                            np.asarray(fc2_b, np.float32),
                            np.asarray(b_ih, np.float32),
                            np.asarray(b_hh, np.float32))
    nc = _CACHE[kh]

    res = run_bass_kernel_spmd(nc, in_maps, list(range(NCORES)))
    h = np.concatenate([res.results[c]["out"] for c in range(NCORES)], axis=0)
    return h[:, None, :].astype(np.float32)
